# revision 1
# baseline (speedup 1.0000x reference)
"""Trainium2 Bass kernel for nn_MetapopLayer (metapopulation SIR scan).

Math: per sample n (1024 total), M=64 locations, C=4 compartments, 100 steps:
    p[n,i]   = 1 - exp(sum_j log(1 - beta*rho[n,i,1]*Rt[n,i,j]/ntot[n,j]))
    q        = R @ p          (per-sample 64x64 matvec)
    new_inf  = (1 - sum_c rho) * q
    rho'     = rho @ T + e0*new_inf, clipped to [0, 1e10]
    trajectory records pre-update rho.

Key device trick: |beta*rho1*Rt/ntot| <= ~0.006, so
p(a) = 1 - exp(-sum_m a^m P_m/m)  (a = rho[n,i,1]) is replaced by a degree-D
polynomial  p(a) = sum_d c_d[n,i] a^d  with coefficients precomputed on host
in float64 (exact to ~1e-10, far below fp32 noise).  The device step is then
pure fp32 tensor ops: Horner (11 small DVE ops), a broadcast-mul + grouped
reduce for the matvec, and a broadcast-mul + grouped reduce for rho@T.

Sharding: pure data-parallel over samples; 128 samples per core on the 128
SBUF partitions.  Raw Bass (Block) implementation — the Tile context's tail
drain trips a sync-wait limit in this walrus build, so semaphores are manual.
"""
import numpy as np

import concourse.bass as bass
from concourse import mybir
from concourse.bass_utils import run_bass_kernel_spmd

F32 = mybir.dt.float32
N, M, C = 1024, 64, 4
TIMESTEPS = 100
NCORES = 8
NS = N // NCORES            # 128 samples per core = SBUF partitions
DEG = 6                     # polynomial degree for p(a)
CLIP_MAX = 1e10


# ----------------------------------------------------------------------
# host-side precompute: polynomial coefficients c_d[n,i]
# ----------------------------------------------------------------------
def _precompute_coeffs(R, beta):
    R64 = R.astype(np.float64)
    ntot = R64.sum(axis=1)                                   # (N, M)
    Rt = np.transpose(R64).reshape(N, M, M)                  # faithful reshape
    V = beta.astype(np.float64)[:, None, None] * Rt / ntot[:, None, :]

    DEG_I = 12   # internal composition degree
    # g(a) = sum_m (P_m/m) a^m
    G = np.zeros((DEG_I + 1, N, M))
    Vp = np.ones_like(V)
    for m in range(1, DEG_I + 1):
        Vp = Vp * V
        G[m] = Vp.sum(axis=2) / m
    # E = exp(-g) as truncated power series;  p = 1 - E
    E = np.zeros((DEG_I + 1, N, M))
    E[0] = 1.0
    Gj = np.zeros((DEG_I + 1, N, M)); Gj[0] = 1.0
    fact = 1.0
    for j in range(1, DEG_I + 1):
        new = np.zeros_like(Gj)
        for d1 in range(j - 1, DEG_I + 1):
            if not Gj[d1].any():
                continue
            for d2 in range(1, DEG_I + 1 - d1):
                new[d1 + d2] += Gj[d1] * G[d2]
        Gj = new
        fact *= j
        E += ((-1) ** j) * Gj / fact
    Cc = -E
    Cc[0] = 0.0
    return Cc[1 : DEG + 1].astype(np.float32)                # (DEG, N, M)


# ----------------------------------------------------------------------
# device kernel builder (per-core program, SPMD across 8 cores)
# ----------------------------------------------------------------------
def _build_bass(run_steps=TIMESTEPS):
    nc = bass.Bass()
    R_d = nc.dram_tensor("R", [NS, M * M], F32, kind="ExternalInput")     # (n,(i,k))
    cd_d = nc.dram_tensor("cd", [NS, DEG * M], F32, kind="ExternalInput")  # (n,(d,i))
    Tb_d = nc.dram_tensor("Tb", [NS, 16], F32, kind="ExternalInput")       # (n,(k,l))
    rho0_d = nc.dram_tensor("rho0", [NS, M * C], F32, kind="ExternalInput")
    traj_d = nc.dram_tensor("traj", [TIMESTEPS, NS, M * C], F32,
                            kind="ExternalOutput")

    mult, add_, mx = mybir.AluOpType.mult, mybir.AluOpType.add, mybir.AluOpType.max

    from contextlib import ExitStack
    with ExitStack() as ctx:
        R_t = ctx.enter_context(nc.sbuf_tensor("R_t", [NS, M * M], F32))
        cd_t = ctx.enter_context(nc.sbuf_tensor("cd_t", [NS, DEG * M], F32))
        Tb_t = ctx.enter_context(nc.sbuf_tensor("Tb_t", [NS, 16], F32))
        rhoA = ctx.enter_context(nc.sbuf_tensor("rhoA", [NS, M * C], F32))
        rhoB = ctx.enter_context(nc.sbuf_tensor("rhoB", [NS, M * C], F32))
        t_mv = ctx.enter_context(nc.sbuf_tensor("t_mv", [NS, M * M], F32))
        Gm = ctx.enter_context(nc.sbuf_tensor("Gm", [NS, M * 16], F32))
        h_t = ctx.enter_context(nc.sbuf_tensor("h_t", [NS, M], F32))
        p_t = ctx.enter_context(nc.sbuf_tensor("p_t", [NS, M], F32))
        q_t = ctx.enter_context(nc.sbuf_tensor("q_t", [NS, M], F32))
        sr_t = ctx.enter_context(nc.sbuf_tensor("sr_t", [NS, M], F32))
        u_t = ctx.enter_context(nc.sbuf_tensor("u_t", [NS, M], F32))
        ni_t = ctx.enter_context(nc.sbuf_tensor("ni_t", [NS, M], F32))
        ones_t = ctx.enter_context(nc.sbuf_tensor("ones_t", [NS, M], F32))
        zero_t = ctx.enter_context(nc.sbuf_tensor("zero_t", [NS, M], F32))
        s_in = ctx.enter_context(nc.semaphore("s_in"))
        s_state = ctx.enter_context(nc.semaphore("s_state"))
        s_out = ctx.enter_context(nc.semaphore("s_out"))
        s_gm = ctx.enter_context(nc.semaphore("s_gm"))
        block = ctx.enter_context(nc.Block())
        s_outB = ctx.enter_context(nc.semaphore("s_outB"))
        rho = [rhoA, rhoB]

        def rho_ap(buf, view):
            base = buf[:].ap[0]
            if view == "a":       # rho[:, 1::4]  (= compartment 1, per i)
                return bass.AP(buf, 1, [base, [4, M]])
            if view == "col0":    # rho[:, 0::4]
                return bass.AP(buf, 0, [base, [4, M]])
            if view == "ic":      # (i, c) for srho reduce
                return bass.AP(buf, 0, [base, [4, M], [1, 4]])
            if view == "G_in":    # (i, l, k): rho[n, i*4+k] bcast over l
                return bass.AP(buf, 0, [base, [4, M], [0, 4], [1, 4]])
            raise ValueError(view)

        @block.sync
        def _(sync):
            sync.dma_start(R_t[:], R_d[:, :]).then_inc(s_in, 16)
            sync.dma_start(cd_t[:], cd_d[:, :]).then_inc(s_in, 16)
            sync.dma_start(Tb_t[:], Tb_d[:, :]).then_inc(s_in, 16)
            sync.dma_start(rhoA[:], rho0_d[:, :]).then_inc(s_in, 16)
            sync.wait_ge(s_in, 64)                  # inputs landed
            H = M * C // 2
            for t in range(run_steps):
                sync.wait_ge(s_state, t)            # rho_t finalized
                dst = bass.AP(traj_d, t * NS * M * C,
                              [[M * C, NS], [1, H]])
                sync.dma_start(dst, rho[t % 2][:, 0:H]).then_inc(s_out, 16)
            sync.wait_ge(s_out, 16 * run_steps)     # all outputs landed
            sync.wait_ge(s_outB, 16 * run_steps)

        @block.scalar
        def _(scalar):
            H = M * C // 2
            scalar.wait_ge(s_in, 64)
            for t in range(run_steps):
                scalar.wait_ge(s_state, t)
                dst = bass.AP(traj_d, t * NS * M * C + H,
                              [[M * C, NS], [1, H]])
                scalar.dma_start(dst, rho[t % 2][:, H:]).then_inc(s_outB, 16)

        @block.gpsimd
        def _(gpsimd):
            # G-mul for step t: Gm[n,(i,l,k)] = rho_t[n,(i,k)] * T[n,(k,l)]
            Tb_bc = bass.AP(Tb_t, 0, [Tb_t[:].ap[0], [0, M], [1, 4], [4, 4]])
            Gm_v = Gm[:].rearrange("n (i l k) -> n i l k", i=M, l=4)
            gpsimd.wait_ge(s_in, 64)
            for t in range(run_steps):
                if t > 0:
                    gpsimd.wait_ge(s_state, t)      # rho_t ready + prev Gm read
                gpsimd.tensor_tensor(out=Gm_v, in0=rho_ap(rho[t % 2], "G_in"),
                                     in1=Tb_bc, op=mult).then_inc(s_gm, 1)

        @block.vector
        def _(vector):
            R_ik = R_t[:].rearrange("n (i k) -> n i k", i=M)
            t_ik = t_mv[:].rearrange("n (i k) -> n i k", i=M)
            p_bc = bass.AP(p_t, 0, [p_t[:].ap[0], [0, M], [1, M]])
            Gm_red = Gm[:].rearrange("n (il k) -> n il k", k=4)
            sub = mybir.AluOpType.subtract
            vector.memset(ones_t[:], 1.0)
            vector.memset(zero_t[:], 0.0)
            vector.wait_ge(s_in, 64)
            for t in range(run_steps):
                cur, nxt = rho[t % 2], rho[(t + 1) % 2]
                a_v = rho_ap(cur, "a")
                # srho, u = 1 - srho (early: consumed several ops later)
                vector.tensor_reduce(out=sr_t[:], in_=rho_ap(cur, "ic"),
                                     axis=mybir.AxisListType.X, op=add_)
                vector.tensor_tensor(out=u_t[:], in0=ones_t[:], in1=sr_t[:], op=sub)
                # p = Horner(c, a)
                vector.tensor_tensor(out=h_t[:], in0=cd_t[:, (DEG - 1) * M : DEG * M],
                                     in1=a_v, op=mult)
                for d in range(DEG - 1, 0, -1):
                    vector.tensor_tensor(out=h_t[:], in0=h_t[:],
                                         in1=cd_t[:, (d - 1) * M : d * M], op=add_)
                    if d > 1:
                        vector.tensor_tensor(out=h_t[:], in0=h_t[:], in1=a_v,
                                             op=mult)
                vector.tensor_tensor(out=p_t[:], in0=h_t[:], in1=a_v, op=mult)
                # q = R @ p  (broadcast-mul + grouped reduce)
                vector.tensor_tensor(out=t_ik, in0=R_ik, in1=p_bc, op=mult)
                vector.tensor_reduce(out=q_t[:], in_=t_ik,
                                     axis=mybir.AxisListType.X, op=add_)
                vector.tensor_tensor(out=ni_t[:], in0=u_t[:], in1=q_t[:], op=mult)
                # rho_next = rho @ T  (+ new_inf into c=0, clip)
                if t > 0:
                    vector.wait_ge(s_out, 16 * t)   # traj[t-1] DMA done
                    vector.wait_ge(s_outB, 16 * t)
                vector.wait_ge(s_gm, t + 1)         # Gm ready
                vector.tensor_reduce(out=nxt[:], in_=Gm_red,
                                     axis=mybir.AxisListType.X, op=add_)
                col0 = rho_ap(nxt, "col0")
                vector.tensor_tensor(out=col0, in0=col0, in1=ni_t[:], op=add_)
                vector.tensor_tensor(out=col0, in0=col0, in1=zero_t[:],
                                     op=mx).then_inc(s_state, 1)
    return nc


_NC_CACHE = None


def kernel(R, T, rho0, beta):
    global _NC_CACHE
    R = np.ascontiguousarray(R, np.float32)
    T = np.ascontiguousarray(T, np.float32)
    rho0 = np.ascontiguousarray(rho0, np.float32)
    beta = np.ascontiguousarray(beta, np.float32)

    cd = _precompute_coeffs(R, beta)                          # (DEG, N, M)
    cd_dev = np.ascontiguousarray(cd.transpose(1, 0, 2)).reshape(N, DEG * M)

    if _NC_CACHE is None:
        _NC_CACHE = _build_bass()
    nc = _NC_CACHE

    in_maps = []
    for c in range(NCORES):
        s = slice(c * NS, (c + 1) * NS)
        in_maps.append({
            "R": R[s].reshape(NS, M * M),
            "cd": cd_dev[s],
            "Tb": T[s].reshape(NS, 16),
            "rho0": rho0[s].reshape(NS, M * C),
        })
    res = run_bass_kernel_spmd(nc, in_maps, core_ids=list(range(NCORES)))
    parts = [r["traj"].reshape(TIMESTEPS, NS, M, C) for r in res.results]
    return np.concatenate(parts, axis=1)



# revision 11
# speedup vs baseline: 828.2984x; 828.2984x over previous
"""Trainium2 Bass kernel for nn_MetapopLayer (metapopulation SIR scan).

Math: per sample n (1024 total), M=64 locations, C=4 compartments, 100 steps:
    p[n,i]   = 1 - exp(sum_j log(1 - beta*rho[n,i,1]*Rt[n,i,j]/ntot[n,j]))
    q        = R @ p          (per-sample 64x64 matvec)
    new_inf  = (1 - sum_c rho) * q
    rho'     = rho @ T + e0*new_inf, clipped to [0, 1e10]
    trajectory records pre-update rho.

Device strategy (per core, 128 samples on the 128 SBUF partitions):
  - p(a) (a = rho[...,1]) is replaced by a degree-2 polynomial with host-
    precomputed (float64) coefficients; degree 2 is converged to ~7e-5 here
    because the log-argument is <= ~0.2*a.
  - The per-sample 64x64 matvec q = R @ p runs on the vector engine in bf16
    (tensor_tensor 2x mode): R (bf16) * broadcast(p), then a bf16 fold tree
    (tensor_reduce only has a 1x uop; the fold tree is ~1.75x faster).
    Whole-trajectory error from bf16 here is ~3.5e-4 (gate: 2e-2).
  - u = 1 - sum_c rho via the exact recurrence u_{t+1} = u_t - ni_t (T is
    row-stochastic, so rho@T preserves mass); computed once at t=0.
  - Engine split: vector does Horner+mul+folds to 128 partials; gpsimd does
    rho@T (broadcast-mul + 2 folds), the last q fold, ni, the col0 update
    and the u recurrence.  The clip is dropped: all terms are provably
    nonnegative and < 1e10.
  - trajectory DMA is split across the sync+scalar queues, double-buffered.

Sharding: pure data-parallel over samples, 128 per core.  Raw Bass (Block)
with manual semaphores (the Tile context's tail drain trips a sync-wait
limit in this walrus build).
"""
import numpy as np

import concourse.bass as bass
from concourse import mybir
from concourse.bass_utils import run_bass_kernel_spmd

F32 = mybir.dt.float32
BF16 = mybir.dt.bfloat16
N, M, C = 1024, 64, 4
TIMESTEPS = 100
NCORES = 8
NS = N // NCORES            # 128 samples per core = SBUF partitions
DEG = 2                     # polynomial degree for p(a)
MM = M * M


# ----------------------------------------------------------------------
# host-side precompute: polynomial coefficients c_d[n,i]
# ----------------------------------------------------------------------
def _precompute_coeffs(R, beta):
    R64 = R.astype(np.float64)
    ntot = R64.sum(axis=1)                                   # (N, M)
    Rt = np.transpose(R64).reshape(N, M, M)                  # faithful reshape
    V = beta.astype(np.float64)[:, None, None] * Rt / ntot[:, None, :]

    DEG_I = 12   # internal composition degree
    # g(a) = sum_m (P_m/m) a^m
    G = np.zeros((DEG_I + 1, N, M))
    Vp = np.ones_like(V)
    for m in range(1, DEG_I + 1):
        Vp = Vp * V
        G[m] = Vp.sum(axis=2) / m
    # E = exp(-g) as truncated power series;  p = 1 - E
    E = np.zeros((DEG_I + 1, N, M))
    E[0] = 1.0
    Gj = np.zeros((DEG_I + 1, N, M)); Gj[0] = 1.0
    fact = 1.0
    for j in range(1, DEG_I + 1):
        new = np.zeros_like(Gj)
        for d1 in range(j - 1, DEG_I + 1):
            if not Gj[d1].any():
                continue
            for d2 in range(1, DEG_I + 1 - d1):
                new[d1 + d2] += Gj[d1] * G[d2]
        Gj = new
        fact *= j
        E += ((-1) ** j) * Gj / fact
    Cc = -E
    Cc[0] = 0.0
    return Cc[1 : DEG + 1].astype(np.float32)                # (DEG, N, M)


def _to_bf16(x):
    """Round-to-nearest-even fp32 -> bf16, kept as uint16 bit pattern."""
    u = np.ascontiguousarray(x, np.float32).view(np.uint32)
    r = ((u + 0x7FFF + ((u >> 16) & 1)) >> 16).astype(np.uint16)
    return r


# ----------------------------------------------------------------------
# device kernel builder (per-core program, SPMD across 8 cores)
# ----------------------------------------------------------------------
def _build_bass(run_steps=TIMESTEPS, traj_len=None):
    if traj_len is None:
        traj_len = run_steps
    nc = bass.Bass()
    R_d = nc.dram_tensor("R", [NS, MM], BF16, kind="ExternalInput")        # (n,(i,k))
    cd_d = nc.dram_tensor("cd", [NS, DEG * M], F32, kind="ExternalInput")  # c1|c2
    Tb_d = nc.dram_tensor("Tb", [NS, 16], F32, kind="ExternalInput")       # (n,(k,l))
    rho0_d = nc.dram_tensor("rho0", [NS, M * C], F32, kind="ExternalInput")
    traj_d = nc.dram_tensor("traj", [traj_len, NS, M * C], F32,
                            kind="ExternalOutput")

    mult, add_ = mybir.AluOpType.mult, mybir.AluOpType.add
    sub = mybir.AluOpType.subtract

    from contextlib import ExitStack
    with ExitStack() as ctx:
        R_t = ctx.enter_context(nc.sbuf_tensor("R_t", [NS, MM], BF16))
        cd_t = ctx.enter_context(nc.sbuf_tensor("cd_t", [NS, DEG * M], F32))
        Tb_t = ctx.enter_context(nc.sbuf_tensor("Tb_t", [NS, 16], F32))
        rhoA = ctx.enter_context(nc.sbuf_tensor("rhoA", [NS, M * C], F32))
        rhoB = ctx.enter_context(nc.sbuf_tensor("rhoB", [NS, M * C], F32))
        t_bf = ctx.enter_context(nc.sbuf_tensor("t_bf", [NS, MM], BF16))
        f1_t = ctx.enter_context(nc.sbuf_tensor("f1_t", [NS, M * 32], BF16))
        f2_t = ctx.enter_context(nc.sbuf_tensor("f2_t", [NS, M * 16], BF16))
        f3_t = ctx.enter_context(nc.sbuf_tensor("f3_t", [NS, M * 8], BF16))
        f4_t = ctx.enter_context(nc.sbuf_tensor("f4_t", [NS, M * 4], BF16))
        f5_t = ctx.enter_context(nc.sbuf_tensor("f5_t", [NS, M * 2], BF16))
        Gm = ctx.enter_context(nc.sbuf_tensor("Gm", [NS, M * 16], F32))
        Gr_t = ctx.enter_context(nc.sbuf_tensor("Gr_t", [NS, M * 8], F32))
        h_t = ctx.enter_context(nc.sbuf_tensor("h_t", [NS, M], F32))
        p_bf = ctx.enter_context(nc.sbuf_tensor("p_bf", [NS, M], BF16))
        q_t = ctx.enter_context(nc.sbuf_tensor("q_t", [NS, M], F32))
        sr_t = ctx.enter_context(nc.sbuf_tensor("sr_t", [NS, M], F32))
        u_t = ctx.enter_context(nc.sbuf_tensor("u_t", [NS, M], F32))
        ni_t = ctx.enter_context(nc.sbuf_tensor("ni_t", [NS, M], F32))
        ones_t = ctx.enter_context(nc.sbuf_tensor("ones_t", [NS, M], F32))
        s_in = ctx.enter_context(nc.semaphore("s_in"))
        s_state = ctx.enter_context(nc.semaphore("s_state"))
        s_out = ctx.enter_context(nc.semaphore("s_out"))
        s_gmr = ctx.enter_context(nc.semaphore("s_gmr"))
        s_q5 = ctx.enter_context(nc.semaphore("s_q5"))
        block = ctx.enter_context(nc.Block())
        s_outB = ctx.enter_context(nc.semaphore("s_outB"))
        rho = [rhoA, rhoB]

        def rho_ap(buf, view):
            base = buf[:].ap[0]
            if view == "a":       # rho[:, 1::4]  (= compartment 1, per i)
                return bass.AP(buf, 1, [base, [4, M]])
            if view == "col0":    # rho[:, 0::4]
                return bass.AP(buf, 0, [base, [4, M]])
            if view == "ic":      # (i, c) for srho reduce
                return bass.AP(buf, 0, [base, [4, M], [1, 4]])
            if view == "G_in":    # (i, l, k): rho[n, i*4+k] bcast over l
                return bass.AP(buf, 0, [base, [4, M], [0, 4], [1, 4]])
            raise ValueError(view)

        H = M * C // 2

        @block.sync
        def _(sync):
            sync.dma_start(R_t[:], R_d[:, :]).then_inc(s_in, 16)
            sync.dma_start(cd_t[:], cd_d[:, :]).then_inc(s_in, 16)
            sync.dma_start(Tb_t[:], Tb_d[:, :]).then_inc(s_in, 16)
            sync.dma_start(rhoA[:], rho0_d[:, :]).then_inc(s_in, 16)
            sync.wait_ge(s_in, 64)                  # inputs landed
            for t in range(run_steps):
                sync.wait_ge(s_state, t)            # rho_t finalized
                dst = bass.AP(traj_d, (t % traj_len) * NS * M * C,
                              [[M * C, NS], [1, H]])
                sync.dma_start(dst, rho[t % 2][:, 0:H]).then_inc(s_out, 16)
            sync.wait_ge(s_out, 16 * run_steps)     # all outputs landed
            sync.wait_ge(s_outB, 16 * run_steps)

        @block.scalar
        def _(scalar):
            scalar.wait_ge(s_in, 64)
            for t in range(run_steps):
                scalar.wait_ge(s_state, t)
                dst = bass.AP(traj_d, (t % traj_len) * NS * M * C + H,
                              [[M * C, NS], [1, H]])
                scalar.dma_start(dst, rho[t % 2][:, H:]).then_inc(s_outB, 16)

        @block.gpsimd
        def _(gpsimd):
            # per step: Gm = rho_t x T (broadcast-mul) + 2 folds -> rho_{t+1};
            # then tail of the q pipeline: last fold, ni, col0 +=, u -= ni
            Tb_bc = bass.AP(Tb_t, 0, [Tb_t[:].ap[0], [0, M], [1, 4], [4, 4]])
            Gm_v = Gm[:].rearrange("n (i l k) -> n i l k", i=M, l=4)
            gb = Gm[:].ap[0]
            grb = Gr_t[:].ap[0]
            f5b = f5_t[:].ap[0]
            gpsimd.wait_ge(s_in, 64)
            for t in range(run_steps):
                cur, nxt = rho[t % 2], rho[(t + 1) % 2]
                gpsimd.tensor_tensor(out=Gm_v, in0=rho_ap(cur, "G_in"),
                                     in1=Tb_bc, op=mult)
                gpsimd.tensor_tensor(
                    out=Gr_t[:].rearrange("n (g k) -> n g k", k=2),
                    in0=bass.AP(Gm, 0, [gb, [4, M * 4], [1, 2]]),
                    in1=bass.AP(Gm, 2, [gb, [4, M * 4], [1, 2]]), op=add_)
                # nxt is the buffer whose traj DMA fired at step t-1
                gpsimd.wait_ge(s_out, 16 * t)
                gpsimd.wait_ge(s_outB, 16 * t)
                gpsimd.tensor_tensor(
                    out=nxt[:],
                    in0=bass.AP(Gr_t, 0, [grb, [2, M * 4]]),
                    in1=bass.AP(Gr_t, 1, [grb, [2, M * 4]]),
                    op=add_).then_inc(s_gmr, 1)
                # q tail: q = f5 even + odd; ni = u*q; col0 += ni; u -= ni
                gpsimd.wait_ge(s_q5, t + 1)
                gpsimd.tensor_tensor(
                    out=q_t[:],
                    in0=bass.AP(f5_t, 0, [f5b, [2, M]]),
                    in1=bass.AP(f5_t, 1, [f5b, [2, M]]), op=add_)
                gpsimd.tensor_tensor(out=ni_t[:], in0=u_t[:], in1=q_t[:],
                                     op=mult)
                col0 = rho_ap(nxt, "col0")
                gpsimd.tensor_tensor(out=col0, in0=col0, in1=ni_t[:],
                                     op=add_).then_inc(s_state, 1)
                gpsimd.tensor_tensor(out=u_t[:], in0=u_t[:], in1=ni_t[:],
                                     op=sub)

        @block.vector
        def _(vector):
            R_ik = R_t[:].rearrange("n (i k) -> n i k", i=M)
            t_ik = t_bf[:].rearrange("n (i k) -> n i k", i=M)
            p_bc = bass.AP(p_bf, 0, [p_bf[:].ap[0], [0, M], [1, M]])

            def fold(dst, src, width, inc=None):
                # dst[:, i, 0:w/2] = src[:, i, 0:w/2] + src[:, i, w/2:w]
                b = src[:].ap[0]
                half = width // 2
                in0 = bass.AP(src, 0, [b, [width, M], [1, half]])
                in1 = bass.AP(src, half, [b, [width, M], [1, half]])
                out = bass.AP(dst, 0, [dst[:].ap[0], [half, M], [1, half]])
                r = vector.tensor_tensor(out=out, in0=in0, in1=in1, op=add_)
                if inc is not None:
                    r.then_inc(inc, 1)

            vector.wait_ge(s_in, 64)
            # u_0 = 1 - sum_c rho0  (then maintained on gpsimd: u -= ni)
            vector.memset(ones_t[:], 1.0)
            vector.tensor_reduce(out=sr_t[:], in_=rho_ap(rhoA, "ic"),
                                 axis=mybir.AxisListType.X, op=add_)
            vector.tensor_tensor(out=u_t[:], in0=ones_t[:], in1=sr_t[:],
                                 op=sub)
            for t in range(run_steps):
                cur = rho[t % 2]
                a_v = rho_ap(cur, "a")
                if t > 0:
                    vector.wait_ge(s_gmr, t)        # rho_t c>=1 cols written
                # p = (c2*a + c1) * a   (degree-2 Horner), output bf16
                vector.tensor_tensor(out=h_t[:], in0=cd_t[:, M : 2 * M],
                                     in1=a_v, op=mult)
                vector.tensor_tensor(out=h_t[:], in0=h_t[:],
                                     in1=cd_t[:, 0:M], op=add_)
                vector.tensor_tensor(out=p_bf[:], in0=h_t[:], in1=a_v, op=mult)
                # q partials: bf16 broadcast-mul + bf16 fold tree to 2 per i
                vector.tensor_tensor(out=t_ik, in0=R_ik, in1=p_bc, op=mult)
                fold(f1_t, t_bf, 64)
                fold(f2_t, f1_t, 32)
                fold(f3_t, f2_t, 16)
                fold(f4_t, f3_t, 8)
                vector.wait_ge(s_state, t)          # f5 reader (fold6) done
                fold(f5_t, f4_t, 4, inc=s_q5)
    return nc


_NC_CACHE = None


def kernel(R, T, rho0, beta):
    global _NC_CACHE
    R = np.ascontiguousarray(R, np.float32)
    T = np.ascontiguousarray(T, np.float32)
    rho0 = np.ascontiguousarray(rho0, np.float32)
    beta = np.ascontiguousarray(beta, np.float32)

    cd = _precompute_coeffs(R, beta)                          # (DEG, N, M)
    # device layout: [c1 | c2] along the free dim
    cd_dev = np.ascontiguousarray(cd.transpose(1, 0, 2)).reshape(N, DEG * M)
    R_bf = _to_bf16(R.reshape(N, MM))                         # uint16 bits

    if _NC_CACHE is None:
        _NC_CACHE = _build_bass()
    nc = _NC_CACHE

    try:
        import ml_dtypes
        R_bf = R_bf.view(ml_dtypes.bfloat16)
    except ImportError:
        pass

    in_maps = []
    for c in range(NCORES):
        s = slice(c * NS, (c + 1) * NS)
        in_maps.append({
            "R": R_bf[s],
            "cd": cd_dev[s],
            "Tb": T[s].reshape(NS, 16),
            "rho0": rho0[s].reshape(NS, M * C),
        })
    res = run_bass_kernel_spmd(nc, in_maps, core_ids=list(range(NCORES)))
    parts = [r["traj"].reshape(TIMESTEPS, NS, M, C) for r in res.results]
    return np.concatenate(parts, axis=1)
